# revision 7
# baseline (speedup 1.0000x reference)
"""Fused attention-encoding kernel for Trainium2, 8-core batch-parallel SPMD.

Problem (per batch b of 16, p=1024 tokens, d=512 features):
    A[i,j] = wa.P_i + wb.P_j + (wc*P_i).P_j        (si = wa.P_i cancels in softmax)
    SA     = softmax_j(A)
    attn   = SA @ P
    Pc     = [P, attn]
    out    = sigmoid(Pc@w2) * P + sigmoid(Pc@w3) * tanh(Pc@w1)

Strategy: batch-parallel over 8 cores (2 batches/core). Per batch, scores are
computed transposed (S^T[j,i], j on partitions) so that
  - sj folds into the exp as a per-partition activation bias,
  - the softmax denominator is a ones-matmul over partitions,
  - the attention matmul consumes E=exp(S^T) directly (no transpose of E),
  - attn^T[d,i] lands exactly in the layout the gate matmuls need as lhsT.
All big matmuls run in bf16 (4x fp32 PE rate); accumulation is fp32 in PSUM.
P is transposed on-chip via DMA-transpose (bf16 xbar path).
"""

import sys

if "/opt/trn_rl_repo" not in sys.path:
    sys.path.insert(0, "/opt/trn_rl_repo")

from contextlib import ExitStack

import ml_dtypes
import numpy as np

import concourse.bass as bass
import concourse.mybir as mybir
import concourse.tile as tile
from concourse import bacc
from concourse.bass_utils import run_bass_kernel_spmd

B, PL, D = 16, 1024, 512
NCORES = 8
BPC = B // NCORES          # batches per core
NI = PL // 128             # token blocks (i or j): 8
ND = D // 128              # feature chunks: 4
NF = 2 * D // 128          # gate contraction chunks: 8
FP32 = mybir.dt.float32
BF16 = mybir.dt.bfloat16
AF = mybir.ActivationFunctionType

_cache = {}


def _build(with_bias: bool, taps: tuple = ()):
    nc = bacc.Bacc(
        "TRN2", target_bir_lowering=False, debug=False, num_devices=1
    )
    p_d = nc.dram_tensor("p_in", [BPC, PL, D], FP32, kind="ExternalInput").ap()
    w_d = nc.dram_tensor("w16", [3, NF, 128, D], BF16, kind="ExternalInput").ap()
    wb_d = nc.dram_tensor("wb16", [ND, 128], BF16, kind="ExternalInput").ap()
    wc_d = nc.dram_tensor("wc32", [ND, 128], FP32, kind="ExternalInput").ap()
    if with_bias:
        b_d = nc.dram_tensor("b32", [3, D], FP32, kind="ExternalInput").ap()
    out_d = nc.dram_tensor("out", [BPC, PL, D], FP32, kind="ExternalOutput").ap()
    tap_d = {}

    with tile.TileContext(nc) as tc, ExitStack() as ctx:
        pool = lambda name, bufs: ctx.enter_context(
            tc.tile_pool(name=name, bufs=bufs)
        )
        const = pool("const", 1)
        wpool = pool("wts", 1)
        pn32p = pool("pn32", 2)
        pn16p = pool("pn16", 2)
        pt16p = pool("pt16", 2)
        pwt16p = pool("pwt16", 2)
        e16p = pool("e16", 2 * NI)
        at16p = pool("at16", 2)
        rb32p = pool("rb32", 2)
        smallp = pool("small", 2)
        gp = pool("gates", 2)
        tmpp = pool("tmp", 2)
        op = pool("outs", 3)
        dramp = ctx.enter_context(tc.tile_pool(name="dram", bufs=2, space="DRAM"))
        psmm = ctx.enter_context(tc.tile_pool(name="psmm", bufs=6, space="PSUM"))
        psvec = ctx.enter_context(tc.tile_pool(name="psvec", bufs=2, space="PSUM"))

        def tap(name, ap, lb=0):
            if lb != 0 or name not in taps:
                return
            t = nc.dram_tensor(
                f"tap_{name}", list(ap.shape), ap.dtype, kind="ExternalOutput"
            ).ap()
            tap_d[name] = t
            nc.sync.dma_start(t, ap)

        # --- constants / weights (once) ---
        w_sb = [
            [wpool.tile([128, D], BF16, tag=f"w{g}_{fc}", name=f"w{g}_{fc}") for fc in range(NF)]
            for g in range(3)
        ]
        for g in range(3):
            for fc in range(NF):
                nc.sync.dma_start(w_sb[g][fc][:], w_d[g, fc])
        wb_sb = const.tile([128, ND], BF16, tag="wb")
        nc.sync.dma_start(wb_sb[:], wb_d.rearrange("c p -> p c"))
        wc_sb = const.tile([128, ND], FP32, tag="wc")
        nc.sync.dma_start(wc_sb[:], wc_d.rearrange("c p -> p c"))
        ones16 = const.tile([128, 1], BF16, tag="ones")
        nc.vector.memset(ones16[:], 1.0)
        if with_bias:
            bb = [const.tile([128, D], FP32, tag=f"bias{g}", name=f"bias{g}") for g in range(3)]
            btmp = const.tile([1, 3 * D], FP32, tag="btmp")
            nc.sync.dma_start(btmp[:], b_d.rearrange("g e -> (g e)")[None, :])
            for g in range(3):
                nc.gpsimd.partition_broadcast(
                    bb[g][:], btmp[0:1, g * D : (g + 1) * D]
                )

        for lb in range(BPC):
            # ---------- phase A: load + prep ----------
            pn32 = pn32p.tile([128, NI * D], FP32, tag="pn32")
            nc.sync.dma_start(
                pn32.rearrange("p (i d) -> p i d", d=D),
                p_d[lb].rearrange("(i p) d -> p i d", p=128),
            )
            pn16 = pn16p.tile([128, NI * D], BF16, tag="pn16")
            for it in range(NI):
                nc.vector.tensor_copy(
                    pn16[:, it * D : (it + 1) * D], pn32[:, it * D : (it + 1) * D]
                )
            pt16 = pt16p.tile([128, ND * PL], BF16, tag="pt16")
            for dc in range(ND):
                for it in range(NI):
                    nc.sync.dma_start(
                        pt16[:, dc * PL + it * 128 : dc * PL + (it + 1) * 128],
                        pn16[:, it * D + dc * 128 : it * D + (dc + 1) * 128],
                        transpose=True,
                    )
            tap("pn16", pn16[:], lb)
            tap("pt16", pt16[:], lb)
            pwt16 = pwt16p.tile([128, ND * PL], BF16, tag="pwt16")
            for dc in range(ND):
                nc.vector.tensor_scalar_mul(
                    pwt16[:, dc * PL : (dc + 1) * PL],
                    pt16[:, dc * PL : (dc + 1) * PL],
                    wc_sb[:, dc : dc + 1],
                )
            # sj[j] = P @ wb, then scatter to per-partition layout [128, NI]
            sj32 = smallp.tile([1, PL], FP32, tag="sj32")
            for jh in range(2):
                ps_sj = psmm.tile([1, 512], FP32, tag="psmm")
                for dc in range(ND):
                    nc.tensor.matmul(
                        ps_sj[:],
                        wb_sb[:, dc : dc + 1],
                        pt16[:, dc * PL + jh * 512 : dc * PL + (jh + 1) * 512],
                        start=(dc == 0),
                        stop=(dc == ND - 1),
                    )
                nc.scalar.copy(sj32[0:1, jh * 512 : (jh + 1) * 512], ps_sj[:])
            # bounce sj through DRAM to redistribute free-dim -> partitions
            sj_dram = dramp.tile([1, PL], FP32, tag="sjdram")
            nc.sync.dma_start(sj_dram[:], sj32[:])
            sjT32 = smallp.tile([128, NI], FP32, tag="sjT32")
            nc.sync.dma_start(
                sjT32[:], sj_dram.rearrange("a (i p) -> (a p) i", p=128)
            )
            tap("pwt16", pwt16[:], lb)
            tap("sj32", sj32[:], lb)
            tap("sjT32", sjT32[:], lb)

            # ---------- phase B: scores + exp + rowsum ----------
            e16 = []
            ps_rs = [psvec.tile([1, 512], FP32, tag="psvec", name=f"psrs{lb}_{_}") for _ in range(2)]
            for jb in range(NI):
                ps_s = [psmm.tile([128, 512], FP32, tag="psmm", name=f"pss{lb}_{jb}_{_}") for _ in range(2)]
                for dc in range(ND):
                    lhsT = pt16[:, dc * PL + jb * 128 : dc * PL + (jb + 1) * 128]
                    for ih in range(2):
                        nc.tensor.matmul(
                            ps_s[ih],
                            lhsT,
                            pwt16[:, dc * PL + ih * 512 : dc * PL + (ih + 1) * 512],
                            start=(dc == 0),
                            stop=(dc == ND - 1),
                        )
                et = e16p.tile([128, PL], BF16, tag="e16")
                e16.append(et)
                for ih in range(2):
                    nc.scalar.activation(
                        et[:, ih * 512 : (ih + 1) * 512],
                        ps_s[ih][:],
                        AF.Exp,
                        bias=sjT32[:, jb : jb + 1],
                    )
                    nc.tensor.matmul(
                        ps_rs[ih][:],
                        ones16[:],
                        et[:, ih * 512 : (ih + 1) * 512],
                        start=(jb == 0),
                        stop=(jb == NI - 1),
                    )
            rr32 = smallp.tile([1, PL], FP32, tag="rr32")
            for ih in range(2):
                nc.vector.reciprocal(
                    rr32[0:1, ih * 512 : (ih + 1) * 512], ps_rs[ih][:]
                )
            rb32 = rb32p.tile([128, PL], FP32, tag="rb32")
            nc.gpsimd.partition_broadcast(rb32[:], rr32[0:1, :])
            tap("e16_0", e16[0][:], lb)
            tap("e16_7", e16[7][:], lb)
            tap("rr32", rr32[:], lb)
            tap("rb32", rb32[:], lb)

            # ---------- phase C: attn^T + normalize ----------
            at16 = at16p.tile([128, ND * PL], BF16, tag="at16")
            for dc in range(ND):
                ps_a = [psmm.tile([128, 512], FP32, tag="psmm", name=f"psa{lb}_{dc}_{_}") for _ in range(2)]
                for jc in range(NI):
                    lhsT = pn16[:, jc * D + dc * 128 : jc * D + (dc + 1) * 128]
                    for ih in range(2):
                        nc.tensor.matmul(
                            ps_a[ih],
                            lhsT,
                            e16[jc][:, ih * 512 : (ih + 1) * 512],
                            start=(jc == 0),
                            stop=(jc == NI - 1),
                        )
                for ih in range(2):
                    nc.vector.tensor_mul(
                        at16[:, dc * PL + ih * 512 : dc * PL + (ih + 1) * 512],
                        ps_a[ih][:],
                        rb32[:, ih * 512 : (ih + 1) * 512],
                    )

            tap("at16", at16[:], lb)
            # ---------- phase D: gates + combine ----------
            for ib in range(NI):
                ps_g = [psmm.tile([128, 512], FP32, tag="psmm", name=f"psg{lb}_{ib}_{_}") for _ in range(3)]
                for fc in range(NF):
                    if fc < ND:
                        lhsT = pt16[:, fc * PL + ib * 128 : fc * PL + (ib + 1) * 128]
                    else:
                        c = fc - ND
                        lhsT = at16[:, c * PL + ib * 128 : c * PL + (ib + 1) * 128]
                    for g in range(3):
                        nc.tensor.matmul(
                            ps_g[g],
                            lhsT,
                            w_sb[g][fc][:],
                            start=(fc == 0),
                            stop=(fc == NF - 1),
                        )
                if with_bias:
                    for g in range(3):
                        nc.vector.tensor_add(ps_g[g][:], ps_g[g][:], bb[g][:])
                z32 = gp.tile([128, D], FP32, tag="z32")
                r32 = gp.tile([128, D], FP32, tag="r32")
                f32 = gp.tile([128, D], FP32, tag="f32")
                nc.scalar.activation(z32[:], ps_g[0][:], AF.Tanh)
                nc.scalar.activation(r32[:], ps_g[1][:], AF.Sigmoid)
                nc.scalar.activation(f32[:], ps_g[2][:], AF.Sigmoid)
                t32 = tmpp.tile([128, D], FP32, tag="t32")
                nc.vector.tensor_mul(t32[:], f32[:], z32[:])
                o32 = op.tile([128, D], FP32, tag="o32")
                nc.vector.tensor_mul(o32[:], r32[:], pn32[:, ib * D : (ib + 1) * D])
                nc.vector.tensor_add(o32[:], o32[:], t32[:])
                nc.sync.dma_start(out_d[lb, ib * 128 : (ib + 1) * 128, :], o32[:])

    nc.compile()
    return nc


def _get_nc(with_bias: bool):
    if with_bias not in _cache:
        _cache[with_bias] = _build(with_bias)
    return _cache[with_bias]


def _prep_in_maps(P, w_atten, w1, w2, w3, b1, b2, b3):
    P = np.ascontiguousarray(np.asarray(P, dtype=np.float32))
    w_atten = np.asarray(w_atten, dtype=np.float32)
    wb = w_atten[D : 2 * D].reshape(ND, 128)
    wc = w_atten[2 * D :].reshape(ND, 128)
    w16 = np.stack(
        [np.asarray(w, dtype=np.float32) for w in (w1, w2, w3)]
    ).reshape(3, NF, 128, D).astype(ml_dtypes.bfloat16)
    biases = np.stack([np.asarray(b, dtype=np.float32) for b in (b1, b2, b3)])
    with_bias = bool(np.any(biases))
    base = {
        "w16": w16,
        "wb16": wb.astype(ml_dtypes.bfloat16),
        "wc32": np.ascontiguousarray(wc),
    }
    if with_bias:
        base["b32"] = biases
    in_maps = []
    for c in range(NCORES):
        m = dict(base)
        m["p_in"] = P[c * BPC : (c + 1) * BPC]
        in_maps.append(m)
    return in_maps, with_bias


def run(P, w_atten, w1, w2, w3, b1, b2, b3, trace=False):
    in_maps, with_bias = _prep_in_maps(P, w_atten, w1, w2, w3, b1, b2, b3)
    nc = _get_nc(with_bias)
    res = run_bass_kernel_spmd(
        nc, in_maps, core_ids=list(range(NCORES)), trace=trace
    )
    out = np.concatenate([res.results[c]["out"] for c in range(NCORES)], axis=0)
    return out, res


def kernel(P, w_atten, w1, w2, w3, b1, b2, b3):
    out, _ = run(P, w_atten, w1, w2, w3, b1, b2, b3)
    return out


# revision 14
# speedup vs baseline: 1.2480x; 1.2480x over previous
"""Fused attention-encoding kernel for Trainium2, 8-core batch-parallel SPMD.

Problem (per batch b of 16, p=1024 tokens, d=512 features):
    A[i,j] = wa.P_i + wb.P_j + (wc*P_i).P_j        (si = wa.P_i cancels in softmax)
    SA     = softmax_j(A)
    attn   = SA @ P
    Pc     = [P, attn]
    out    = sigmoid(Pc@w2) * P + sigmoid(Pc@w3) * tanh(Pc@w1)

Strategy: batch-parallel over 8 cores (2 batches/core). Per batch, scores are
computed transposed (S^T[j,i], j on partitions) so that
  - sj folds into the exp as a per-partition activation bias,
  - the softmax denominator is a ones-matmul over partitions,
  - the attention matmul consumes E=exp(S^T) directly (no transpose of E),
  - attn^T[d,i] lands exactly in the layout the gate matmuls need as lhsT.
All big matmuls run in bf16 (4x fp32 PE rate); accumulation is fp32 in PSUM.
P is transposed on-chip via DMA-transpose (bf16 xbar path).
"""

import sys

if "/opt/trn_rl_repo" not in sys.path:
    sys.path.insert(0, "/opt/trn_rl_repo")

from contextlib import ExitStack

import ml_dtypes
import numpy as np

import concourse.bass as bass
import concourse.mybir as mybir
import concourse.tile as tile
from concourse import bacc
from concourse.bass_utils import run_bass_kernel_spmd

B, PL, D = 16, 1024, 512
NCORES = 8
BPC = B // NCORES          # batches per core
NI = PL // 128             # token blocks (i or j): 8
ND = D // 128              # feature chunks: 4
NF = 2 * D // 128          # gate contraction chunks: 8
FP32 = mybir.dt.float32
BF16 = mybir.dt.bfloat16
AF = mybir.ActivationFunctionType

_cache = {}


def _build(with_bias: bool, taps: tuple = ()):
    nc = bacc.Bacc(
        "TRN2", target_bir_lowering=False, debug=False, num_devices=1
    )
    p_d = nc.dram_tensor("p_in", [BPC, PL, D], FP32, kind="ExternalInput").ap()
    p16_d = nc.dram_tensor("p16", [BPC, PL, D], BF16, kind="ExternalInput").ap()
    w_d = nc.dram_tensor("w16", [3, NF, 128, D], BF16, kind="ExternalInput").ap()
    wb_d = nc.dram_tensor("wb16", [ND, 128], BF16, kind="ExternalInput").ap()
    wc_d = nc.dram_tensor("wc32", [ND, 128], FP32, kind="ExternalInput").ap()
    if with_bias:
        b_d = nc.dram_tensor("b32", [3, D], FP32, kind="ExternalInput").ap()
    out_d = nc.dram_tensor("out", [BPC, PL, D], FP32, kind="ExternalOutput").ap()
    tap_d = {}

    with tile.TileContext(nc) as tc, ExitStack() as ctx:
        pool = lambda name, bufs: ctx.enter_context(
            tc.tile_pool(name=name, bufs=bufs)
        )
        const = pool("const", 1)
        wpool = pool("wts", 1)
        pn32p = pool("pn32", 2)
        pn16p = pool("pn16", 2)
        pt16p = pool("pt16", 2)
        pwt16p = pool("pwt16", 2)
        e16p = pool("e16", 2 * NI)
        at16p = pool("at16", 2)
        rb32p = pool("rb32", 2)
        smallp = pool("small", 2)
        gp = pool("gates", 2)
        tmpp = pool("tmp", 2)
        op = pool("outs", 3)
        dramp = ctx.enter_context(tc.tile_pool(name="dram", bufs=2, space="DRAM"))
        psmm = ctx.enter_context(tc.tile_pool(name="psmm", bufs=6, space="PSUM"))
        psvec = ctx.enter_context(tc.tile_pool(name="psvec", bufs=2, space="PSUM"))

        def tap(name, ap, lb=0):
            if lb != 0 or name not in taps:
                return
            t = nc.dram_tensor(
                f"tap_{name}", list(ap.shape), ap.dtype, kind="ExternalOutput"
            ).ap()
            tap_d[name] = t
            nc.sync.dma_start(t, ap)

        # --- constants / weights (once) ---
        w_sb = [
            [wpool.tile([128, D], BF16, tag=f"w{g}_{fc}", name=f"w{g}_{fc}") for fc in range(NF)]
            for g in range(3)
        ]
        for g in range(3):
            for fc in range(NF):
                nc.gpsimd.dma_start(w_sb[g][fc][:], w_d[g, fc])
        wb_sb = const.tile([128, ND], BF16, tag="wb")
        nc.scalar.dma_start(wb_sb[:], wb_d.rearrange("c p -> p c"))
        wc_sb = const.tile([128, ND], FP32, tag="wc")
        nc.scalar.dma_start(wc_sb[:], wc_d.rearrange("c p -> p c"))
        ones16 = const.tile([128, 1], BF16, tag="ones")
        nc.vector.memset(ones16[:], 1.0)
        if with_bias:
            bb = [const.tile([128, D], FP32, tag=f"bias{g}", name=f"bias{g}") for g in range(3)]
            btmp = const.tile([1, 3 * D], FP32, tag="btmp")
            nc.sync.dma_start(btmp[:], b_d.rearrange("g e -> (g e)")[None, :])
            for g in range(3):
                nc.gpsimd.partition_broadcast(
                    bb[g][:], btmp[0:1, g * D : (g + 1) * D]
                )

        for lb in range(BPC):
            # ---------- phase A: load + prep ----------
            pn32 = pn32p.tile([128, NI * D], FP32, tag="pn32")
            nc.sync.dma_start(
                pn32.rearrange("p (i d) -> p i d", d=D),
                p_d[lb].rearrange("(i p) d -> p i d", p=128),
            )
            pn16 = pn16p.tile([128, NI * D], BF16, tag="pn16")
            nc.sync.dma_start(
                pn16.rearrange("p (i d) -> p i d", d=D),
                p16_d[lb].rearrange("(i p) d -> p i d", p=128),
            )
            pt16 = pt16p.tile([128, ND * PL], BF16, tag="pt16")
            for dc in range(ND):
                nc.sync.dma_start(
                    pt16[:, dc * PL : (dc + 1) * PL],
                    p16_d[lb][:, dc * 128 : (dc + 1) * 128],
                    transpose=True,
                )
            tap("pn16", pn16[:], lb)
            tap("pt16", pt16[:], lb)
            pwt16 = pwt16p.tile([128, ND * PL], BF16, tag="pwt16")
            for dc in range(ND):
                nc.vector.tensor_scalar_mul(
                    pwt16[:, dc * PL : (dc + 1) * PL],
                    pt16[:, dc * PL : (dc + 1) * PL],
                    wc_sb[:, dc : dc + 1],
                )
            # sj[j] = P @ wb, then scatter to per-partition layout [128, NI]
            sj32 = smallp.tile([1, PL], FP32, tag="sj32")
            for jh in range(2):
                ps_sj = psmm.tile([1, 512], FP32, tag="psmm")
                for dc in range(ND):
                    nc.tensor.matmul(
                        ps_sj[:],
                        wb_sb[:, dc : dc + 1],
                        pt16[:, dc * PL + jh * 512 : dc * PL + (jh + 1) * 512],
                        start=(dc == 0),
                        stop=(dc == ND - 1),
                    )
                nc.scalar.copy(sj32[0:1, jh * 512 : (jh + 1) * 512], ps_sj[:])
            # bounce sj through DRAM to redistribute free-dim -> partitions
            sj_dram = dramp.tile([1, PL], FP32, tag="sjdram")
            nc.scalar.dma_start(sj_dram[:], sj32[:])
            sjT32 = smallp.tile([128, NI], FP32, tag="sjT32")
            nc.scalar.dma_start(
                sjT32[:], sj_dram.rearrange("a (i p) -> (a p) i", p=128)
            )
            tap("pwt16", pwt16[:], lb)
            tap("sj32", sj32[:], lb)
            tap("sjT32", sjT32[:], lb)

            # ---------- phase B: scores + exp + rowsum ----------
            e16 = []
            ps_rs = [psvec.tile([1, 512], FP32, tag="psvec", name=f"psrs{lb}_{_}") for _ in range(2)]
            for jb in range(NI):
                ps_s = [psmm.tile([128, 512], FP32, tag="psmm", name=f"pss{lb}_{jb}_{_}") for _ in range(2)]
                for dc in range(ND):
                    lhsT = pt16[:, dc * PL + jb * 128 : dc * PL + (jb + 1) * 128]
                    for ih in range(2):
                        nc.tensor.matmul(
                            ps_s[ih],
                            lhsT,
                            pwt16[:, dc * PL + ih * 512 : dc * PL + (ih + 1) * 512],
                            start=(dc == 0),
                            stop=(dc == ND - 1),
                        )
                et = e16p.tile([128, PL], BF16, tag="e16")
                e16.append(et)
                for ih in range(2):
                    nc.scalar.activation(
                        et[:, ih * 512 : (ih + 1) * 512],
                        ps_s[ih][:],
                        AF.Exp,
                        bias=sjT32[:, jb : jb + 1],
                    )
                    nc.tensor.matmul(
                        ps_rs[ih][:],
                        ones16[:],
                        et[:, ih * 512 : (ih + 1) * 512],
                        start=(jb == 0),
                        stop=(jb == NI - 1),
                    )
            rs32 = smallp.tile([1, PL], FP32, tag="rs32")
            for ih in range(2):
                nc.scalar.copy(rs32[0:1, ih * 512 : (ih + 1) * 512], ps_rs[ih][:])
            rsb32 = rb32p.tile([128, PL], FP32, tag="rsb32", bufs=1)
            nc.gpsimd.partition_broadcast(rsb32[:], rs32[0:1, :])
            rb32 = rb32p.tile([128, PL], FP32, tag="rb32")
            nc.vector.reciprocal(rb32[:], rsb32[:])
            tap("e16_0", e16[0][:], lb)
            tap("e16_7", e16[7][:], lb)
            tap("rr32", rb32[0:1, :], lb)
            tap("rb32", rb32[:], lb)

            # ---------- phase C: attn^T + normalize ----------
            at16 = at16p.tile([128, ND * PL], BF16, tag="at16")
            for dc in range(ND):
                ps_a = [psmm.tile([128, 512], FP32, tag="psmm", name=f"psa{lb}_{dc}_{_}") for _ in range(2)]
                for jc in range(NI):
                    lhsT = pn16[:, jc * D + dc * 128 : jc * D + (dc + 1) * 128]
                    for ih in range(2):
                        nc.tensor.matmul(
                            ps_a[ih],
                            lhsT,
                            e16[jc][:, ih * 512 : (ih + 1) * 512],
                            start=(jc == 0),
                            stop=(jc == NI - 1),
                        )
                for ih in range(2):
                    nc.vector.tensor_mul(
                        at16[:, dc * PL + ih * 512 : dc * PL + (ih + 1) * 512],
                        ps_a[ih][:],
                        rb32[:, ih * 512 : (ih + 1) * 512],
                    )

            tap("at16", at16[:], lb)
            # ---------- phase D: gates + combine ----------
            for ib in range(NI):
                ps_g = [psmm.tile([128, 512], FP32, tag="psmm", name=f"psg{lb}_{ib}_{_}") for _ in range(3)]
                for fc in range(NF):
                    if fc < ND:
                        lhsT = pt16[:, fc * PL + ib * 128 : fc * PL + (ib + 1) * 128]
                    else:
                        c = fc - ND
                        lhsT = at16[:, c * PL + ib * 128 : c * PL + (ib + 1) * 128]
                    for g in range(3):
                        nc.tensor.matmul(
                            ps_g[g],
                            lhsT,
                            w_sb[g][fc][:],
                            start=(fc == 0),
                            stop=(fc == NF - 1),
                        )
                if with_bias:
                    for g in range(3):
                        nc.vector.tensor_add(ps_g[g][:], ps_g[g][:], bb[g][:])
                z32 = gp.tile([128, D], FP32, tag="z32")
                r32 = gp.tile([128, D], FP32, tag="r32")
                f32 = gp.tile([128, D], FP32, tag="f32")
                nc.scalar.activation(z32[:], ps_g[0][:], AF.Tanh)
                nc.scalar.activation(r32[:], ps_g[1][:], AF.Sigmoid)
                nc.scalar.activation(f32[:], ps_g[2][:], AF.Sigmoid)
                t32 = tmpp.tile([128, D], FP32, tag="t32")
                nc.vector.tensor_mul(t32[:], f32[:], z32[:])
                o32 = op.tile([128, D], FP32, tag="o32")
                nc.vector.tensor_mul(o32[:], r32[:], pn32[:, ib * D : (ib + 1) * D])
                nc.vector.tensor_add(o32[:], o32[:], t32[:])
                nc.sync.dma_start(out_d[lb, ib * 128 : (ib + 1) * 128, :], o32[:])

    nc.compile()
    return nc


def _get_nc(with_bias: bool):
    if with_bias not in _cache:
        _cache[with_bias] = _build(with_bias)
    return _cache[with_bias]


def _prep_in_maps(P, w_atten, w1, w2, w3, b1, b2, b3):
    P = np.ascontiguousarray(np.asarray(P, dtype=np.float32))
    w_atten = np.asarray(w_atten, dtype=np.float32)
    wb = w_atten[D : 2 * D].reshape(ND, 128)
    wc = w_atten[2 * D :].reshape(ND, 128)
    w16 = np.stack(
        [np.asarray(w, dtype=np.float32) for w in (w1, w2, w3)]
    ).reshape(3, NF, 128, D).astype(ml_dtypes.bfloat16)
    biases = np.stack([np.asarray(b, dtype=np.float32) for b in (b1, b2, b3)])
    with_bias = bool(np.any(biases))
    P16 = P.astype(ml_dtypes.bfloat16)
    base = {
        "w16": w16,
        "wb16": wb.astype(ml_dtypes.bfloat16),
        "wc32": np.ascontiguousarray(wc),
    }
    if with_bias:
        base["b32"] = biases
    in_maps = []
    for c in range(NCORES):
        m = dict(base)
        m["p_in"] = P[c * BPC : (c + 1) * BPC]
        m["p16"] = P16[c * BPC : (c + 1) * BPC]
        in_maps.append(m)
    return in_maps, with_bias


def run(P, w_atten, w1, w2, w3, b1, b2, b3, trace=False):
    in_maps, with_bias = _prep_in_maps(P, w_atten, w1, w2, w3, b1, b2, b3)
    nc = _get_nc(with_bias)
    res = run_bass_kernel_spmd(
        nc, in_maps, core_ids=list(range(NCORES)), trace=trace
    )
    out = np.concatenate([res.results[c]["out"] for c in range(NCORES)], axis=0)
    return out, res


def kernel(P, w_atten, w1, w2, w3, b1, b2, b3):
    out, _ = run(P, w_atten, w1, w2, w3, b1, b2, b3)
    return out


# revision 17
# speedup vs baseline: 1.4635x; 1.1727x over previous
"""Fused attention-encoding kernel for Trainium2, 8-core batch-parallel SPMD.

Problem (per batch b of 16, p=1024 tokens, d=512 features):
    A[i,j] = wa.P_i + wb.P_j + (wc*P_i).P_j        (si = wa.P_i cancels in softmax)
    SA     = softmax_j(A)
    attn   = SA @ P
    Pc     = [P, attn]
    out    = sigmoid(Pc@w2) * P + sigmoid(Pc@w3) * tanh(Pc@w1)

Strategy: batch-parallel over 8 cores (2 batches/core). Per batch, scores are
computed transposed (S^T[j,i], j on partitions) so that
  - sj folds into the exp as a per-partition activation bias,
  - the softmax denominator is a ones-matmul over partitions,
  - the attention matmul consumes E=exp(S^T) directly (no transpose of E),
  - attn^T[d,i] lands exactly in the layout the gate matmuls need as lhsT.
All big matmuls run in bf16 (4x fp32 PE rate); accumulation is fp32 in PSUM.
P is transposed on-chip via DMA-transpose (bf16 xbar path).
"""

import sys

if "/opt/trn_rl_repo" not in sys.path:
    sys.path.insert(0, "/opt/trn_rl_repo")

from contextlib import ExitStack

import ml_dtypes
import numpy as np

import concourse.bass as bass
import concourse.mybir as mybir
import concourse.tile as tile
from concourse import bacc
from concourse.bass_utils import run_bass_kernel_spmd

B, PL, D = 16, 1024, 512
NCORES = 8
BPC = B // NCORES          # batches per core
NI = PL // 128             # token blocks (i or j): 8
ND = D // 128              # feature chunks: 4
NF = 2 * D // 128          # gate contraction chunks: 8
FP32 = mybir.dt.float32
BF16 = mybir.dt.bfloat16
AF = mybir.ActivationFunctionType

_cache = {}


def _build(with_bias: bool, taps: tuple = ()):
    nc = bacc.Bacc(
        "TRN2", target_bir_lowering=False, debug=False, num_devices=1
    )
    p_d = nc.dram_tensor("p_in", [BPC, PL, D], FP32, kind="ExternalInput").ap()
    p16_d = nc.dram_tensor("p16", [BPC, PL, D], BF16, kind="ExternalInput").ap()
    w_d = nc.dram_tensor("w16", [3, NF, 128, D], BF16, kind="ExternalInput").ap()
    wb_d = nc.dram_tensor("wb16", [ND, 128], BF16, kind="ExternalInput").ap()
    wc_d = nc.dram_tensor("wc32", [ND, 128], FP32, kind="ExternalInput").ap()
    if with_bias:
        b_d = nc.dram_tensor("b32", [3, D], FP32, kind="ExternalInput").ap()
    out_d = nc.dram_tensor("out", [BPC, PL, D], FP32, kind="ExternalOutput").ap()
    tap_d = {}

    with tile.TileContext(nc) as tc, ExitStack() as ctx:
        pool = lambda name, bufs: ctx.enter_context(
            tc.tile_pool(name=name, bufs=bufs)
        )
        const = pool("const", 1)
        wpool = pool("wts", 1)
        pn32p = pool("pn32", 2)
        pn16p = pool("pn16", 2)
        pt16p = pool("pt16", 2)
        pwt16p = pool("pwt16", 2)
        e16p = pool("e16", 2 * NI)
        at16p = pool("at16", 2)
        rb32p = pool("rb32", 2)
        smallp = pool("small", 2)
        gp = pool("gates", 2)
        tmpp = pool("tmp", 2)
        op = pool("outs", 3)
        dramp = ctx.enter_context(tc.tile_pool(name="dram", bufs=2, space="DRAM"))
        psmm = ctx.enter_context(tc.tile_pool(name="psmm", bufs=6, space="PSUM"))
        psvec = ctx.enter_context(tc.tile_pool(name="psvec", bufs=2, space="PSUM"))

        def tap(name, ap, lb=0):
            if lb != 0 or name not in taps:
                return
            t = nc.dram_tensor(
                f"tap_{name}", list(ap.shape), ap.dtype, kind="ExternalOutput"
            ).ap()
            tap_d[name] = t
            nc.sync.dma_start(t, ap)

        # --- constants / weights (once) ---
        w_sb = [
            [wpool.tile([128, D], BF16, tag=f"w{g}_{fc}", name=f"w{g}_{fc}") for fc in range(NF)]
            for g in range(3)
        ]
        def load_weights():
            # issued on the sync ring *after* batch-0's critical loads so the
            # FIFO gives the scores path full HBM bandwidth first
            for g in range(3):
                for fc in range(NF):
                    nc.sync.dma_start(w_sb[g][fc][:], w_d[g, fc])
        wb_sb = const.tile([128, ND], BF16, tag="wb")
        nc.scalar.dma_start(wb_sb[:], wb_d.rearrange("c p -> p c"))
        wc_sb = const.tile([128, ND], FP32, tag="wc")
        nc.scalar.dma_start(wc_sb[:], wc_d.rearrange("c p -> p c"))
        ones16 = const.tile([128, 1], BF16, tag="ones")
        nc.vector.memset(ones16[:], 1.0)
        if with_bias:
            bb = [const.tile([128, D], FP32, tag=f"bias{g}", name=f"bias{g}") for g in range(3)]
            btmp = const.tile([1, 3 * D], FP32, tag="btmp")
            nc.sync.dma_start(btmp[:], b_d.rearrange("g e -> (g e)")[None, :])
            for g in range(3):
                nc.gpsimd.partition_broadcast(
                    bb[g][:], btmp[0:1, g * D : (g + 1) * D]
                )

        for lb in range(BPC):
            # ---------- phase A: load + prep ----------
            # sync-ring order = HBM priority: transposes (scores path) first,
            # then pn16 (attn), then weights (gates, batch 0 only), then pn32
            # (final combine).
            pt16 = pt16p.tile([128, ND * PL], BF16, tag="pt16")
            for dc in range(ND):
                nc.sync.dma_start(
                    pt16[:, dc * PL : (dc + 1) * PL],
                    p16_d[lb][:, dc * 128 : (dc + 1) * 128],
                    transpose=True,
                )
            pn16 = pn16p.tile([128, NI * D], BF16, tag="pn16")
            nc.sync.dma_start(
                pn16.rearrange("p (i d) -> p i d", d=D),
                p16_d[lb].rearrange("(i p) d -> p i d", p=128),
            )
            if lb == 0:
                load_weights()
            pn32 = pn32p.tile([128, NI * D], FP32, tag="pn32")
            nc.sync.dma_start(
                pn32.rearrange("p (i d) -> p i d", d=D),
                p_d[lb].rearrange("(i p) d -> p i d", p=128),
            )
            tap("pn16", pn16[:], lb)
            tap("pt16", pt16[:], lb)
            pwt16 = pwt16p.tile([128, ND * PL], BF16, tag="pwt16")
            for dc in range(ND):
                nc.vector.tensor_scalar_mul(
                    pwt16[:, dc * PL : (dc + 1) * PL],
                    pt16[:, dc * PL : (dc + 1) * PL],
                    wc_sb[:, dc : dc + 1],
                )
            # sj[j] = P @ wb, then scatter to per-partition layout [128, NI]
            sj32 = smallp.tile([1, PL], FP32, tag="sj32")
            for jh in range(2):
                ps_sj = psmm.tile([1, 512], FP32, tag="psmm")
                for dc in range(ND):
                    nc.tensor.matmul(
                        ps_sj[:],
                        wb_sb[:, dc : dc + 1],
                        pt16[:, dc * PL + jh * 512 : dc * PL + (jh + 1) * 512],
                        start=(dc == 0),
                        stop=(dc == ND - 1),
                    )
                nc.scalar.copy(sj32[0:1, jh * 512 : (jh + 1) * 512], ps_sj[:])
            # bounce sj through DRAM to redistribute free-dim -> partitions
            sj_dram = dramp.tile([1, PL], FP32, tag="sjdram")
            nc.scalar.dma_start(sj_dram[:], sj32[:])
            sjT32 = smallp.tile([128, NI], FP32, tag="sjT32")
            nc.scalar.dma_start(
                sjT32[:], sj_dram.rearrange("a (i p) -> (a p) i", p=128)
            )
            tap("pwt16", pwt16[:], lb)
            tap("sj32", sj32[:], lb)
            tap("sjT32", sjT32[:], lb)

            # ---------- phase B: scores + exp + rowsum ----------
            e16 = []
            ps_rs = [psvec.tile([1, 512], FP32, tag="psvec", name=f"psrs{lb}_{_}") for _ in range(2)]
            for jb in range(NI):
                ps_s = [psmm.tile([128, 512], FP32, tag="psmm", name=f"pss{lb}_{jb}_{_}") for _ in range(2)]
                for dc in range(ND):
                    lhsT = pt16[:, dc * PL + jb * 128 : dc * PL + (jb + 1) * 128]
                    for ih in range(2):
                        nc.tensor.matmul(
                            ps_s[ih],
                            lhsT,
                            pwt16[:, dc * PL + ih * 512 : dc * PL + (ih + 1) * 512],
                            start=(dc == 0),
                            stop=(dc == ND - 1),
                        )
                et = e16p.tile([128, PL], BF16, tag="e16")
                e16.append(et)
                for ih in range(2):
                    nc.scalar.activation(
                        et[:, ih * 512 : (ih + 1) * 512],
                        ps_s[ih][:],
                        AF.Exp,
                        bias=sjT32[:, jb : jb + 1],
                    )
                    nc.tensor.matmul(
                        ps_rs[ih][:],
                        ones16[:],
                        et[:, ih * 512 : (ih + 1) * 512],
                        start=(jb == 0),
                        stop=(jb == NI - 1),
                    )
            rs32 = smallp.tile([1, PL], FP32, tag="rs32")
            for ih in range(2):
                nc.scalar.copy(rs32[0:1, ih * 512 : (ih + 1) * 512], ps_rs[ih][:])
            rsb32 = rb32p.tile([128, PL], FP32, tag="rsb32", bufs=1)
            nc.gpsimd.partition_broadcast(rsb32[:], rs32[0:1, :])
            rb32 = rb32p.tile([128, PL], FP32, tag="rb32")
            nc.vector.reciprocal_approx_fast(out=rb32[:], in_=rsb32[:])
            tap("e16_0", e16[0][:], lb)
            tap("e16_7", e16[7][:], lb)
            tap("rr32", rb32[0:1, :], lb)
            tap("rb32", rb32[:], lb)

            # ---------- phase C: attn^T + normalize ----------
            at16 = at16p.tile([128, ND * PL], BF16, tag="at16")
            for dc in range(ND):
                ps_a = [psmm.tile([128, 512], FP32, tag="psmm", name=f"psa{lb}_{dc}_{_}") for _ in range(2)]
                for jc in range(NI):
                    lhsT = pn16[:, jc * D + dc * 128 : jc * D + (dc + 1) * 128]
                    for ih in range(2):
                        nc.tensor.matmul(
                            ps_a[ih],
                            lhsT,
                            e16[jc][:, ih * 512 : (ih + 1) * 512],
                            start=(jc == 0),
                            stop=(jc == NI - 1),
                        )
                for ih in range(2):
                    nc.vector.tensor_mul(
                        at16[:, dc * PL + ih * 512 : dc * PL + (ih + 1) * 512],
                        ps_a[ih][:],
                        rb32[:, ih * 512 : (ih + 1) * 512],
                    )

            tap("at16", at16[:], lb)
            # ---------- phase D: gates + combine ----------
            for ib in range(NI):
                ps_g = [psmm.tile([128, 512], FP32, tag="psmm", name=f"psg{lb}_{ib}_{_}") for _ in range(3)]
                for fc in range(NF):
                    if fc < ND:
                        lhsT = pt16[:, fc * PL + ib * 128 : fc * PL + (ib + 1) * 128]
                    else:
                        c = fc - ND
                        lhsT = at16[:, c * PL + ib * 128 : c * PL + (ib + 1) * 128]
                    for g in range(3):
                        nc.tensor.matmul(
                            ps_g[g],
                            lhsT,
                            w_sb[g][fc][:],
                            start=(fc == 0),
                            stop=(fc == NF - 1),
                        )
                if with_bias:
                    for g in range(3):
                        nc.vector.tensor_add(ps_g[g][:], ps_g[g][:], bb[g][:])
                z32 = gp.tile([128, D], FP32, tag="z32")
                r32 = gp.tile([128, D], FP32, tag="r32")
                f32 = gp.tile([128, D], FP32, tag="f32")
                nc.scalar.activation(z32[:], ps_g[0][:], AF.Tanh)
                nc.scalar.activation(r32[:], ps_g[1][:], AF.Sigmoid)
                nc.scalar.activation(f32[:], ps_g[2][:], AF.Sigmoid)
                t32 = tmpp.tile([128, D], FP32, tag="t32")
                nc.vector.tensor_mul(t32[:], f32[:], z32[:])
                o32 = op.tile([128, D], FP32, tag="o32")
                nc.vector.tensor_mul(o32[:], r32[:], pn32[:, ib * D : (ib + 1) * D])
                nc.vector.tensor_add(o32[:], o32[:], t32[:])
                nc.sync.dma_start(out_d[lb, ib * 128 : (ib + 1) * 128, :], o32[:])

    nc.compile()
    return nc


def _get_nc(with_bias: bool):
    if with_bias not in _cache:
        _cache[with_bias] = _build(with_bias)
    return _cache[with_bias]


def _prep_in_maps(P, w_atten, w1, w2, w3, b1, b2, b3):
    P = np.ascontiguousarray(np.asarray(P, dtype=np.float32))
    w_atten = np.asarray(w_atten, dtype=np.float32)
    wb = w_atten[D : 2 * D].reshape(ND, 128)
    wc = w_atten[2 * D :].reshape(ND, 128)
    w16 = np.stack(
        [np.asarray(w, dtype=np.float32) for w in (w1, w2, w3)]
    ).reshape(3, NF, 128, D).astype(ml_dtypes.bfloat16)
    biases = np.stack([np.asarray(b, dtype=np.float32) for b in (b1, b2, b3)])
    with_bias = bool(np.any(biases))
    P16 = P.astype(ml_dtypes.bfloat16)
    base = {
        "w16": w16,
        "wb16": wb.astype(ml_dtypes.bfloat16),
        "wc32": np.ascontiguousarray(wc),
    }
    if with_bias:
        base["b32"] = biases
    in_maps = []
    for c in range(NCORES):
        m = dict(base)
        m["p_in"] = P[c * BPC : (c + 1) * BPC]
        m["p16"] = P16[c * BPC : (c + 1) * BPC]
        in_maps.append(m)
    return in_maps, with_bias


def run(P, w_atten, w1, w2, w3, b1, b2, b3, trace=False):
    in_maps, with_bias = _prep_in_maps(P, w_atten, w1, w2, w3, b1, b2, b3)
    nc = _get_nc(with_bias)
    res = run_bass_kernel_spmd(
        nc, in_maps, core_ids=list(range(NCORES)), trace=trace
    )
    out = np.concatenate([res.results[c]["out"] for c in range(NCORES)], axis=0)
    return out, res


def kernel(P, w_atten, w1, w2, w3, b1, b2, b3):
    out, _ = run(P, w_atten, w1, w2, w3, b1, b2, b3)
    return out


# revision 18
# speedup vs baseline: 1.4922x; 1.0196x over previous
"""Fused attention-encoding kernel for Trainium2, 8-core batch-parallel SPMD.

Problem (per batch b of 16, p=1024 tokens, d=512 features):
    A[i,j] = wa.P_i + wb.P_j + (wc*P_i).P_j        (si = wa.P_i cancels in softmax)
    SA     = softmax_j(A)
    attn   = SA @ P
    Pc     = [P, attn]
    out    = sigmoid(Pc@w2) * P + sigmoid(Pc@w3) * tanh(Pc@w1)

Strategy: batch-parallel over 8 cores (2 batches/core). Per batch, scores are
computed transposed (S^T[j,i], j on partitions) so that
  - sj folds into the exp as a per-partition activation bias,
  - the softmax denominator is a ones-matmul over partitions,
  - the attention matmul consumes E=exp(S^T) directly (no transpose of E),
  - attn^T[d,i] lands exactly in the layout the gate matmuls need as lhsT.
All big matmuls run in bf16 (4x fp32 PE rate); accumulation is fp32 in PSUM.
P is transposed on-chip via DMA-transpose (bf16 xbar path).
"""

import sys

if "/opt/trn_rl_repo" not in sys.path:
    sys.path.insert(0, "/opt/trn_rl_repo")

from contextlib import ExitStack

import ml_dtypes
import numpy as np

import concourse.bass as bass
import concourse.mybir as mybir
import concourse.tile as tile
from concourse import bacc
from concourse.bass_utils import run_bass_kernel_spmd

B, PL, D = 16, 1024, 512
NCORES = 8
BPC = B // NCORES          # batches per core
NI = PL // 128             # token blocks (i or j): 8
ND = D // 128              # feature chunks: 4
NF = 2 * D // 128          # gate contraction chunks: 8
FP32 = mybir.dt.float32
BF16 = mybir.dt.bfloat16
AF = mybir.ActivationFunctionType

_cache = {}


def _build(with_bias: bool, taps: tuple = ()):
    nc = bacc.Bacc(
        "TRN2", target_bir_lowering=False, debug=False, num_devices=1
    )
    p_d = nc.dram_tensor("p_in", [BPC, PL, D], FP32, kind="ExternalInput").ap()
    p16_d = nc.dram_tensor("p16", [BPC, PL, D], BF16, kind="ExternalInput").ap()
    w_d = nc.dram_tensor("w16", [3, NF, 128, D], BF16, kind="ExternalInput").ap()
    wb_d = nc.dram_tensor("wb16", [ND, 128], BF16, kind="ExternalInput").ap()
    wc_d = nc.dram_tensor("wc32", [ND, 128], FP32, kind="ExternalInput").ap()
    if with_bias:
        b_d = nc.dram_tensor("b32", [3, D], FP32, kind="ExternalInput").ap()
    out_d = nc.dram_tensor("out", [BPC, PL, D], FP32, kind="ExternalOutput").ap()
    tap_d = {}

    with tile.TileContext(nc) as tc, ExitStack() as ctx:
        pool = lambda name, bufs: ctx.enter_context(
            tc.tile_pool(name=name, bufs=bufs)
        )
        const = pool("const", 1)
        wpool = pool("wts", 1)
        pn32p = pool("pn32", 2)
        pn16p = pool("pn16", 2)
        pt16p = pool("pt16", 2)
        pwt16p = pool("pwt16", 2)
        e16p = pool("e16", 2 * NI)
        at16p = pool("at16", 2)
        rb32p = pool("rb32", 2)
        smallp = pool("small", 2)
        gp = pool("gates", 2)
        tmpp = pool("tmp", 2)
        op = pool("outs", 3)
        dramp = ctx.enter_context(tc.tile_pool(name="dram", bufs=2, space="DRAM"))
        psmm = ctx.enter_context(tc.tile_pool(name="psmm", bufs=6, space="PSUM"))
        psvec = ctx.enter_context(tc.tile_pool(name="psvec", bufs=2, space="PSUM"))

        def tap(name, ap, lb=0):
            if lb != 0 or name not in taps:
                return
            t = nc.dram_tensor(
                f"tap_{name}", list(ap.shape), ap.dtype, kind="ExternalOutput"
            ).ap()
            tap_d[name] = t
            nc.sync.dma_start(t, ap)

        # --- constants / weights (once) ---
        w_sb = [
            [wpool.tile([128, D], BF16, tag=f"w{g}_{fc}", name=f"w{g}_{fc}") for fc in range(NF)]
            for g in range(3)
        ]
        def load_weights():
            # issued on the sync ring *after* batch-0's critical loads so the
            # FIFO gives the scores path full HBM bandwidth first
            for g in range(3):
                for fc in range(NF):
                    nc.sync.dma_start(w_sb[g][fc][:], w_d[g, fc])
        wb_sb = const.tile([128, ND], BF16, tag="wb")
        nc.scalar.dma_start(wb_sb[:], wb_d.rearrange("c p -> p c"))
        wc_sb = const.tile([128, ND], FP32, tag="wc")
        nc.scalar.dma_start(wc_sb[:], wc_d.rearrange("c p -> p c"))
        ones16 = const.tile([128, 1], BF16, tag="ones")
        nc.vector.memset(ones16[:], 1.0)
        ones_row = const.tile([1, 512], BF16, tag="ones_row")
        nc.vector.memset(ones_row[:], 1.0)
        if with_bias:
            bb = [const.tile([128, D], FP32, tag=f"bias{g}", name=f"bias{g}") for g in range(3)]
            btmp = const.tile([1, 3 * D], FP32, tag="btmp")
            nc.sync.dma_start(btmp[:], b_d.rearrange("g e -> (g e)")[None, :])
            for g in range(3):
                nc.gpsimd.partition_broadcast(
                    bb[g][:], btmp[0:1, g * D : (g + 1) * D]
                )

        for lb in range(BPC):
            # ---------- phase A: load + prep ----------
            # sync-ring order = HBM priority: transposes (scores path) first,
            # then pn16 (attn), then weights (gates, batch 0 only), then pn32
            # (final combine).
            pt16 = pt16p.tile([128, ND * PL], BF16, tag="pt16")
            for dc in range(ND):
                nc.sync.dma_start(
                    pt16[:, dc * PL : (dc + 1) * PL],
                    p16_d[lb][:, dc * 128 : (dc + 1) * 128],
                    transpose=True,
                )
            pn16 = pn16p.tile([128, NI * D], BF16, tag="pn16")
            nc.sync.dma_start(
                pn16.rearrange("p (i d) -> p i d", d=D),
                p16_d[lb].rearrange("(i p) d -> p i d", p=128),
            )
            if lb == 0:
                load_weights()
            pn32 = pn32p.tile([128, NI * D], FP32, tag="pn32")
            nc.sync.dma_start(
                pn32.rearrange("p (i d) -> p i d", d=D),
                p_d[lb].rearrange("(i p) d -> p i d", p=128),
            )
            tap("pn16", pn16[:], lb)
            tap("pt16", pt16[:], lb)
            pwt16 = pwt16p.tile([128, ND * PL], BF16, tag="pwt16")
            for dc in range(ND):
                nc.vector.tensor_scalar_mul(
                    pwt16[:, dc * PL : (dc + 1) * PL],
                    pt16[:, dc * PL : (dc + 1) * PL],
                    wc_sb[:, dc : dc + 1],
                )
            # sj[j] = P @ wb as a bf16 row; folded into scores as a
            # K=1 rank-1 update (sj_col x ones_row) so exp has no bias dep
            sj16 = smallp.tile([1, PL], BF16, tag="sj16")
            for jh in range(2):
                ps_sj = psvec.tile([1, 512], FP32, tag="psvec", name=f"pssj{lb}_{jh}")
                for dc in range(ND):
                    nc.tensor.matmul(
                        ps_sj[:],
                        wb_sb[:, dc : dc + 1],
                        pt16[:, dc * PL + jh * 512 : dc * PL + (jh + 1) * 512],
                        start=(dc == 0),
                        stop=(dc == ND - 1),
                    )
                nc.scalar.copy(sj16[0:1, jh * 512 : (jh + 1) * 512], ps_sj[:])
            tap("pwt16", pwt16[:], lb)

            # ---------- phase B: scores + exp + rowsum ----------
            e16 = []
            ps_rs = [psvec.tile([1, 512], FP32, tag="psvec", name=f"psrs{lb}_{_}") for _ in range(2)]
            for jb in range(NI):
                ps_s = [psmm.tile([128, 512], FP32, tag="psmm", name=f"pss{lb}_{jb}_{_}") for _ in range(2)]
                for dc in range(ND):
                    lhsT = pt16[:, dc * PL + jb * 128 : dc * PL + (jb + 1) * 128]
                    for ih in range(2):
                        nc.tensor.matmul(
                            ps_s[ih],
                            lhsT,
                            pwt16[:, dc * PL + ih * 512 : dc * PL + (ih + 1) * 512],
                            start=(dc == 0),
                            stop=False,
                        )
                for ih in range(2):
                    nc.tensor.matmul(
                        ps_s[ih],
                        sj16[0:1, jb * 128 : (jb + 1) * 128],
                        ones_row[:],
                        start=False,
                        stop=True,
                    )
                et = e16p.tile([128, PL], BF16, tag="e16")
                e16.append(et)
                for ih in range(2):
                    nc.scalar.activation(
                        et[:, ih * 512 : (ih + 1) * 512],
                        ps_s[ih][:],
                        AF.Exp,
                    )
                    nc.tensor.matmul(
                        ps_rs[ih][:],
                        ones16[:],
                        et[:, ih * 512 : (ih + 1) * 512],
                        start=(jb == 0),
                        stop=(jb == NI - 1),
                    )
            rs32 = smallp.tile([1, PL], FP32, tag="rs32")
            for ih in range(2):
                nc.scalar.copy(rs32[0:1, ih * 512 : (ih + 1) * 512], ps_rs[ih][:])
            rsb32 = rb32p.tile([128, PL], FP32, tag="rsb32", bufs=1)
            nc.gpsimd.partition_broadcast(rsb32[:], rs32[0:1, :])
            rb32 = rb32p.tile([128, PL], FP32, tag="rb32")
            nc.vector.reciprocal_approx_fast(out=rb32[:], in_=rsb32[:])
            tap("e16_0", e16[0][:], lb)
            tap("e16_7", e16[7][:], lb)
            tap("rr32", rb32[0:1, :], lb)
            tap("rb32", rb32[:], lb)

            # ---------- phase C: attn^T + normalize ----------
            at16 = at16p.tile([128, ND * PL], BF16, tag="at16")
            for dc in range(ND):
                ps_a = [psmm.tile([128, 512], FP32, tag="psmm", name=f"psa{lb}_{dc}_{_}") for _ in range(2)]
                for jc in range(NI):
                    lhsT = pn16[:, jc * D + dc * 128 : jc * D + (dc + 1) * 128]
                    for ih in range(2):
                        nc.tensor.matmul(
                            ps_a[ih],
                            lhsT,
                            e16[jc][:, ih * 512 : (ih + 1) * 512],
                            start=(jc == 0),
                            stop=(jc == NI - 1),
                        )
                for ih in range(2):
                    nc.vector.tensor_mul(
                        at16[:, dc * PL + ih * 512 : dc * PL + (ih + 1) * 512],
                        ps_a[ih][:],
                        rb32[:, ih * 512 : (ih + 1) * 512],
                    )

            tap("at16", at16[:], lb)
            # ---------- phase D: gates + combine ----------
            for ib in range(NI):
                ps_g = [psmm.tile([128, 512], FP32, tag="psmm", name=f"psg{lb}_{ib}_{_}") for _ in range(3)]
                for fc in range(NF):
                    if fc < ND:
                        lhsT = pt16[:, fc * PL + ib * 128 : fc * PL + (ib + 1) * 128]
                    else:
                        c = fc - ND
                        lhsT = at16[:, c * PL + ib * 128 : c * PL + (ib + 1) * 128]
                    for g in range(3):
                        nc.tensor.matmul(
                            ps_g[g],
                            lhsT,
                            w_sb[g][fc][:],
                            start=(fc == 0),
                            stop=(fc == NF - 1),
                        )
                if with_bias:
                    for g in range(3):
                        nc.vector.tensor_add(ps_g[g][:], ps_g[g][:], bb[g][:])
                z32 = gp.tile([128, D], FP32, tag="z32")
                r32 = gp.tile([128, D], FP32, tag="r32")
                f32 = gp.tile([128, D], FP32, tag="f32")
                nc.scalar.activation(z32[:], ps_g[0][:], AF.Tanh)
                nc.scalar.activation(r32[:], ps_g[1][:], AF.Sigmoid)
                nc.scalar.activation(f32[:], ps_g[2][:], AF.Sigmoid)
                t32 = tmpp.tile([128, D], FP32, tag="t32")
                nc.vector.tensor_mul(t32[:], f32[:], z32[:])
                o32 = op.tile([128, D], FP32, tag="o32")
                nc.vector.tensor_mul(o32[:], r32[:], pn32[:, ib * D : (ib + 1) * D])
                nc.vector.tensor_add(o32[:], o32[:], t32[:])
                nc.sync.dma_start(out_d[lb, ib * 128 : (ib + 1) * 128, :], o32[:])

    nc.compile()
    return nc


def _get_nc(with_bias: bool):
    if with_bias not in _cache:
        _cache[with_bias] = _build(with_bias)
    return _cache[with_bias]


def _prep_in_maps(P, w_atten, w1, w2, w3, b1, b2, b3):
    P = np.ascontiguousarray(np.asarray(P, dtype=np.float32))
    w_atten = np.asarray(w_atten, dtype=np.float32)
    wb = w_atten[D : 2 * D].reshape(ND, 128)
    wc = w_atten[2 * D :].reshape(ND, 128)
    w16 = np.stack(
        [np.asarray(w, dtype=np.float32) for w in (w1, w2, w3)]
    ).reshape(3, NF, 128, D).astype(ml_dtypes.bfloat16)
    biases = np.stack([np.asarray(b, dtype=np.float32) for b in (b1, b2, b3)])
    with_bias = bool(np.any(biases))
    P16 = P.astype(ml_dtypes.bfloat16)
    base = {
        "w16": w16,
        "wb16": wb.astype(ml_dtypes.bfloat16),
        "wc32": np.ascontiguousarray(wc),
    }
    if with_bias:
        base["b32"] = biases
    in_maps = []
    for c in range(NCORES):
        m = dict(base)
        m["p_in"] = P[c * BPC : (c + 1) * BPC]
        m["p16"] = P16[c * BPC : (c + 1) * BPC]
        in_maps.append(m)
    return in_maps, with_bias


def run(P, w_atten, w1, w2, w3, b1, b2, b3, trace=False):
    in_maps, with_bias = _prep_in_maps(P, w_atten, w1, w2, w3, b1, b2, b3)
    nc = _get_nc(with_bias)
    res = run_bass_kernel_spmd(
        nc, in_maps, core_ids=list(range(NCORES)), trace=trace
    )
    out = np.concatenate([res.results[c]["out"] for c in range(NCORES)], axis=0)
    return out, res


def kernel(P, w_atten, w1, w2, w3, b1, b2, b3):
    out, _ = run(P, w_atten, w1, w2, w3, b1, b2, b3)
    return out
